# revision 1
# baseline (speedup 1.0000x reference)
"""Trainium2 Bass kernel for nn_NeuralNetworkDPD (dense_mlp).

Strategy (feature-major, 2-token-halves packed on 128 partitions):
  - Each core handles 4 batch rows. A-half = rows {0,1}, B-half = rows {2,3},
    packed as SBUF partitions [0:64)=A-token features, [64:128)=B-token feats.
  - Dense layers: block-diag(W, W) stationary [128,128]; each streamed column
    carries 2 tokens -> 0.5 PE cycles/token/layer.
  - LayerNorm stats as broadcast PLANES: a block-diag(ones/64) stationary
    reduces over the feature partitions and replicates the result to all 64
    output partitions of each half, so mean/var arrive already broadcast:
        mu_bc  = onesd @ z      (one matmul)
        var_bc = onesd @ (z-mu_bc)^2
  - Normalize: v=(z-mu_bc); rs=recip_approx(sqrt(var_bc+eps)); u=Prelu on
    ScalarE fusing gamma (scale), beta (bias), alpha - all per-partition.
  - skip connection and b_out applied host-side (cheap rank-1/elementwise).
"""

import sys
from contextlib import ExitStack

sys.path.insert(0, "/opt/trn_rl_repo")

import numpy as np

import concourse.bacc as bacc
import concourse.bass as bass
import concourse.tile as tile
from concourse import mybir

F = 64          # feature width
NL = 6          # chained dense layers
EPS = 1e-3
CH = 512        # tokens per matmul (PSUM bank)
SUP = 8         # chunks per super-chunk (scheduling window)
R = mybir.dt.float32r   # dtype of all matmul-feeding tensors (1 cyc/row)


def build_kernel(tc, outs, ins, tokens_per_row):
    """Emit the Tile program. ins/outs are dicts of DRAM APs."""
    nc = tc.nc
    TPR = tokens_per_row
    cpr = TPR // CH              # chunks per row
    spr = cpr // SUP             # super-chunks per row
    assert cpr % SUP == 0
    NG = SUP // 2                # groups (of 2 chunks) per super

    xr, xi = ins["xr"], ins["xi"]
    out = outs["out"]            # [4, TPR, 2] fp32

    # Internal padded copies of x: [4, TPR+3], first 3 entries zero.
    xpad_r = nc.dram_tensor("xpad_r", [4, TPR + 3], R,
                            kind="Internal").ap()
    xpad_i = nc.dram_tensor("xpad_i", [4, TPR + 3], R,
                            kind="Internal").ap()

    ctx = ExitStack()
    singles = ctx.enter_context(tc.tile_pool(name="singles", bufs=1))
    zpool = ctx.enter_context(tc.tile_pool(name="zpool", bufs=8))
    rpool = ctx.enter_context(tc.tile_pool(name="rpool", bufs=8))
    upool = ctx.enter_context(tc.tile_pool(name="upool", bufs=3))
    vpool = ctx.enter_context(tc.tile_pool(name="vpool", bufs=4))
    qpool = ctx.enter_context(tc.tile_pool(name="qpool", bufs=4))
    fpool = ctx.enter_context(tc.tile_pool(name="fpool", bufs=4))
    opool = ctx.enter_context(tc.tile_pool(name="opool", bufs=3))
    zp_pool = ctx.enter_context(tc.tile_pool(name="zp", bufs=2, space="PSUM"))
    mu_pool = ctx.enter_context(tc.tile_pool(name="mu", bufs=2, space="PSUM"))
    va_pool = ctx.enter_context(tc.tile_pool(name="va", bufs=2, space="PSUM"))

    # ---- load weights/constants into SBUF ----
    wd = singles.tile([128, NL * 128], R)
    win = singles.tile([16, 128], R)
    wout = singles.tile([128, 4], R)
    onesd = singles.tile([128, 128], R)
    percol = singles.tile([128, 25], mybir.dt.float32)
    epsc = singles.tile([128, 1], mybir.dt.float32)
    nc.sync.dma_start(out=wd, in_=ins["wd"])
    nc.sync.dma_start(out=win, in_=ins["win"])
    nc.sync.dma_start(out=wout, in_=ins["wout"])
    nc.sync.dma_start(out=onesd, in_=ins["onesd"])
    nc.sync.dma_start(out=percol, in_=ins["percol"])
    nc.vector.memset(epsc, EPS)

    b_in_col = percol[:, 0:1]
    dense_b_col = [percol[:, 1 + l: 2 + l] for l in range(NL)]
    gamma_col = [percol[:, 7 + l: 8 + l] for l in range(NL)]
    beta_col = [percol[:, 13 + l: 14 + l] for l in range(NL)]
    alpha_col = [percol[:, 19 + l: 20 + l] for l in range(NL)]

    # ---- build zero-padded x in DRAM ----
    zrow = singles.tile([1, 4], R)
    nc.vector.memset(zrow.bitcast(mybir.dt.float32), 0.0)
    for r in range(4):
        for xp in (xpad_r, xpad_i):
            nc.sync.dma_start(out=xp[r: r + 1, 0:3], in_=zrow[0:1, 0:3])
    nc.sync.dma_start(out=xpad_r[:, 3:], in_=xr)
    nc.sync.dma_start(out=xpad_i[:, 3:], in_=xi)

    # ---------------- main loops ----------------
    for rp in range(2):                     # row-pair: A=row rp, B=row 2+rp
        rowA, rowB = rp, 2 + rp
        for s in range(spr):                # super-chunk
            # -- w_in: windowed feats + first dense for 8 chunks --
            zps = []                        # psum tiles holding current z
            for k in range(SUP):
                t0 = (s * SUP + k) * CH
                feats = fpool.tile([16, CH], R, tag="feats")
                # A-half lags: rows 0-3 real, 4-7 imag; B-half: rows 8-15
                for (base, row) in ((0, rowA), (8, rowB)):
                    src_r = bass.AP(tensor=xpad_r.tensor,
                                    offset=row * (TPR + 3) + t0,
                                    ap=[[1, 4], [1, CH]])
                    src_i = bass.AP(tensor=xpad_i.tensor,
                                    offset=row * (TPR + 3) + t0,
                                    ap=[[1, 4], [1, CH]])
                    nc.sync.dma_start(out=feats[base: base + 4, :], in_=src_r)
                    nc.sync.dma_start(out=feats[base + 4: base + 8, :], in_=src_i)
                if k % 2 == 0:
                    zp = zp_pool.tile([128, 2 * CH], mybir.dt.float32, tag="zp")
                    zps.append(zp)
                nc.tensor.matmul(out=zps[-1][:, (k % 2) * CH:(k % 2 + 1) * CH],
                                 lhsT=(win[:, :]), rhs=(feats),
                                 start=True, stop=True)

            res = [None, None, None]        # z0, z2, z4 anchor groups
            z_groups = [None] * NG

            for l in range(NL + 1):         # 6 LN+PReLU+dense stages + final
                bias = b_in_col if l == 0 else dense_b_col[l - 1]
                new_z = [None] * NG
                for g in range(NG):
                    if l in (0, 2, 4):
                        zt = rpool.tile([128, 2 * CH], R,
                                        tag="za", name=f"za{l}g{g}")
                    else:
                        zt = zpool.tile([128, 2 * CH], R,
                                        tag="z", name=f"z{l}g{g}")
                    nc.scalar.activation(out=zt, in_=zps[g],
                                         func=mybir.ActivationFunctionType.Identity,
                                         bias=bias, scale=1.0)
                    if l in (2, 4, 6):      # residual add at block boundaries
                        if l == 6:
                            zsum = zpool.tile([128, 2 * CH], R,
                                              tag="z", name=f"zs{l}g{g}")
                        else:
                            zsum = rpool.tile([128, 2 * CH], R,
                                              tag="zb", name=f"zs{l}g{g}")
                        nc.vector.tensor_add(zsum, zt, res[l // 2 - 1][g])
                        zt = zsum
                    new_z[g] = zt
                z_groups = new_z
                if l in (0, 2, 4):
                    res[l // 2] = z_groups
                if l == NL:
                    break

                zps = []
                for g in range(NG):
                    zg = z_groups[g]
                    u = upool.tile([128, 2 * CH], R, tag="u")
                    for j in range(2):
                        zsl = zg[:, j * CH:(j + 1) * CH]
                        # mean plane (already broadcast to both halves)
                        mu = mu_pool.tile([128, CH], mybir.dt.float32, tag="mu")
                        nc.tensor.matmul(out=mu, lhsT=(onesd[:, :]),
                                         rhs=(zsl), start=True, stop=True)
                        v = vpool.tile([128, CH], mybir.dt.float32, tag="v")
                        nc.vector.tensor_sub(v, zsl, mu)
                        vsq = qpool.tile([128, CH], R, tag="vsq")
                        nc.scalar.activation(
                            out=vsq, in_=v,
                            func=mybir.ActivationFunctionType.Square)
                        va = va_pool.tile([128, CH], mybir.dt.float32, tag="va")
                        nc.tensor.matmul(out=va, lhsT=(onesd[:, :]),
                                         rhs=(vsq), start=True, stop=True)
                        sg = qpool.tile([128, CH], mybir.dt.float32, tag="sg")
                        nc.scalar.activation(
                            out=sg, in_=va,
                            func=mybir.ActivationFunctionType.Sqrt,
                            bias=epsc, scale=1.0)
                        rs = vpool.tile([128, CH], mybir.dt.float32, tag="rs")
                        nc.vector.reciprocal_approx_fast(out=rs, in_=sg)
                        nc.vector.tensor_mul(u[:, j * CH:(j + 1) * CH], v, rs)
                    # PReLU(gamma*x + beta) fused on ScalarE, in place on u
                    nc.scalar.activation(out=u, in_=u,
                                         func=mybir.ActivationFunctionType.Prelu,
                                         bias=beta_col[l], scale=gamma_col[l],
                                         alpha=alpha_col[l])
                    zp = zp_pool.tile([128, 2 * CH], mybir.dt.float32, tag="zp")
                    for j in range(2):
                        nc.tensor.matmul(
                            out=zp[:, j * CH:(j + 1) * CH],
                            lhsT=(wd[:, l * 128:(l + 1) * 128]),
                            rhs=(u[:, j * CH:(j + 1) * CH]),
                            start=True, stop=True)
                    zps.append(zp)

            # -- w_out + store --
            for g in range(NG):
                for j in range(2):
                    k = 2 * g + j
                    t0 = (s * SUP + k) * CH
                    op = mu_pool.tile([4, CH], mybir.dt.float32, tag="mu",
                                      padded_shape=[128, CH])
                    nc.tensor.matmul(out=op, lhsT=(wout[:, :]),
                                     rhs=(z_groups[g][:, j * CH:(j + 1) * CH]),
                                     start=True, stop=True)
                    ot = opool.tile([4, CH], mybir.dt.float32, tag="ot")
                    nc.scalar.copy(out=ot, in_=op)
                    for (half, row) in ((0, rowA), (1, rowB)):
                        dst = bass.AP(tensor=out.tensor,
                                      offset=row * TPR * 2 + t0 * 2,
                                      ap=[[1, 2], [2, CH]])
                        nc.sync.dma_start(out=dst,
                                          in_=ot[2 * half: 2 * half + 2, :])
    ctx.close()


def _host_pack(inputs):
    """Build the shared (replicated) packed-weight arrays."""
    w_in = np.asarray(inputs["w_in"], np.float32)
    dense_w = np.asarray(inputs["dense_w"], np.float32)
    w_out = np.asarray(inputs["w_out"], np.float32)
    ln_gamma = np.asarray(inputs["ln_gamma"], np.float32)
    ln_beta = np.asarray(inputs["ln_beta"], np.float32)
    alpha = np.asarray(inputs["alpha"], np.float32)
    b_in = np.asarray(inputs["b_in"], np.float32)
    dense_b = np.asarray(inputs["dense_b"], np.float32)

    wd = np.zeros((128, NL * 128), np.float32)
    for l in range(NL):
        wd[0:64, l * 128: l * 128 + 64] = dense_w[l]
        wd[64:128, l * 128 + 64: l * 128 + 128] = dense_w[l]
    win = np.zeros((16, 128), np.float32)
    win[0:8, 0:64] = w_in
    win[8:16, 64:128] = w_in
    wout = np.zeros((128, 4), np.float32)
    wout[0:64, 0:2] = w_out
    wout[64:128, 2:4] = w_out
    onesd = np.zeros((128, 128), np.float32)
    onesd[0:64, 0:64] = 1.0 / F
    onesd[64:128, 64:128] = 1.0 / F
    percol = np.zeros((128, 25), np.float32)
    percol[:, 0] = np.tile(b_in, 2)
    for l in range(NL):
        percol[:, 1 + l] = np.tile(dense_b[l], 2)
        percol[:, 7 + l] = np.tile(ln_gamma[l], 2)
        percol[:, 13 + l] = np.tile(ln_beta[l], 2)
        percol[:, 19 + l] = np.tile(alpha[l], 2)
    return dict(wd=wd, win=win, wout=wout, onesd=onesd, percol=percol)


def build_program(tokens_per_row):
    """Build the full Bass/Tile program for one core's shard."""
    nc = bacc.Bacc("TRN2")
    ins = {}
    shapes = dict(wd=(128, NL * 128), win=(16, 128), wout=(128, 4),
                  onesd=(128, 128), percol=(128, 25))
    for name, shp in shapes.items():
        dt = mybir.dt.float32 if name == "percol" else R
        ins[name] = nc.dram_tensor(name, list(shp), dt,
                                   kind="ExternalInput").ap()
    ins["xr"] = nc.dram_tensor("xr", [4, tokens_per_row], R,
                               kind="ExternalInput").ap()
    ins["xi"] = nc.dram_tensor("xi", [4, tokens_per_row], R,
                               kind="ExternalInput").ap()
    outs = {"out": nc.dram_tensor("out", [4, tokens_per_row, 2],
                                  mybir.dt.float32, kind="ExternalOutput").ap()}
    with tile.TileContext(nc) as tc:
        build_kernel(tc, outs, ins, tokens_per_row)
    nc.compile()
    return nc


def _run(inputs, trace=False):
    from concourse.bass_utils import run_bass_kernel_spmd

    x_real = np.asarray(inputs["x_real"], np.float32)
    x_imag = np.asarray(inputs["x_imag"], np.float32)
    B, N = x_real.shape
    n_cores = 8
    rows_per_core = B // n_cores

    shared = _host_pack(inputs)
    nc = build_program(N)

    in_maps = []
    for c in range(n_cores):
        m = dict(shared)
        m["xr"] = np.ascontiguousarray(x_real[c * rows_per_core:(c + 1) * rows_per_core])
        m["xi"] = np.ascontiguousarray(x_imag[c * rows_per_core:(c + 1) * rows_per_core])
        in_maps.append(m)

    res = run_bass_kernel_spmd(nc, in_maps, core_ids=list(range(n_cores)),
                               trace=trace)
    outs_np = [r["out"] for r in res.results]
    full = np.concatenate(outs_np, axis=0)          # [B, N, 2]
    b_out = np.asarray(inputs["b_out"], np.float32)
    re = full[..., 0] + b_out[0] + x_real
    im = full[..., 1] + b_out[1] + x_imag
    return (re + 1j * im).astype(np.complex64), res


def kernel(**inputs):
    return _run(inputs, trace=False)[0]



# revision 5
# speedup vs baseline: 1.3368x; 1.3368x over previous
"""Trainium2 Bass kernel for nn_NeuralNetworkDPD (dense_mlp) — v3.

Design ("selective centering + prelu decomposition", bf16):
  - LayerNorm-only stages (1,3,5): dense weights column-centered host-side
    (W' = W - rowmean), so the matmul output IS the centered value v and
    no mean computation is needed at all.
  - Residual-boundary stages (0,2,4): weights kept uncentered so the TRUE
    z' (needed for the residual chain and the final w_out) materializes;
    centering for LN happens on the fly: one block-diag ones/64 matmul
    makes the broadcast mean plane, one DVE subtract gives v.
  - Residual adds run on the PE: identity-stationary matmuls accumulate
    the TRUE source into the next boundary stage's PSUM.
  - var = E[v^2] via ones/64 matmul on v^2 (broadcast plane);
    rs = 1/sqrt(var+eps) in ONE ScalarE op (Abs_reciprocal_sqrt). All ACT
    funcs used live in one table set -> no table switching.
  - PReLU decomposed: u = (1-a).relu(y) + a.y, y = gamma*x + beta. The
    next dense runs TWO matmuls (Wa on r=max(y,0), Wb on y) with alphas
    folded into the stationaries; y/r are cheap DVE tensor_scalar ops.
  - Final stage output z6' is TRUE, so out = w_out.T z6' directly — no
    mean-correction machinery.
  - bf16 everywhere on SBUF; DMAs batched (2 in + 2 out per group).
"""

import sys
from contextlib import ExitStack

sys.path.insert(0, "/opt/trn_rl_repo")

import numpy as np

import concourse.bacc as bacc
import concourse.bass as bass
import concourse.tile as tile
from concourse import mybir

F = 64          # feature width
NL = 6          # chained dense layers
EPS = 1e-3
CH = 512        # tokens per matmul (PSUM bank)
SUP = 8         # chunks per super-chunk (scheduling window)
GW = 2 * CH     # group width (2 chunks per group)
R = mybir.dt.float32r
BF = mybir.dt.bfloat16
F32 = mybir.dt.float32
AF = mybir.ActivationFunctionType
ALU = mybir.AluOpType
TRUE_STAGES = (0, 2, 4)     # stages producing TRUE (uncentered) values


def build_kernel(tc, outs, ins, tokens_per_row):
    nc = tc.nc
    TPR = tokens_per_row
    cpr = TPR // CH
    spr = cpr // SUP
    assert cpr % SUP == 0
    NG = SUP // 2               # groups (of 2 chunks) per super

    xr, xi = ins["xr"], ins["xi"]
    out = outs["out"]           # [4, TPR, 2] fp32

    xpad_r = nc.dram_tensor("xpad_r", [4, TPR + 3], R, kind="Internal").ap()
    xpad_i = nc.dram_tensor("xpad_i", [4, TPR + 3], R, kind="Internal").ap()

    ctx = ExitStack()
    singles = ctx.enter_context(tc.tile_pool(name="singles", bufs=1))
    vbsp = ctx.enter_context(tc.tile_pool(name="vbs", bufs=8))    # TRUE src
    vbtp = ctx.enter_context(tc.tile_pool(name="vbt", bufs=8))    # v tiles
    sqp = ctx.enter_context(tc.tile_pool(name="sq", bufs=8))
    rsp = ctx.enter_context(tc.tile_pool(name="rs", bufs=8))
    xhp = ctx.enter_context(tc.tile_pool(name="xh", bufs=8))
    ytp = ctx.enter_context(tc.tile_pool(name="ytr", bufs=8))     # y/r pairs
    ftp = ctx.enter_context(tc.tile_pool(name="feats", bufs=9))   # fp32r
    otp = ctx.enter_context(tc.tile_pool(name="ot", bufs=4))      # fp32 [4,GW]
    zpp = ctx.enter_context(tc.tile_pool(name="zp", bufs=2, space="PSUM"))
    vap = ctx.enter_context(tc.tile_pool(name="va", bufs=2, space="PSUM"))
    mup = ctx.enter_context(tc.tile_pool(name="mu", bufs=2, space="PSUM"))

    # ---- weights/constants ----
    win = singles.tile([16, 128], R)
    wa = singles.tile([128, NL * 128], BF)
    wb = singles.tile([128, NL * 128], BF)
    onesd = singles.tile([128, 128], BF)
    ident = singles.tile([128, 128], BF)
    wout = singles.tile([128, 4], BF)
    percol = singles.tile([128, 19], F32)  # b x7, gamma x6, beta x6
    epsc = singles.tile([128, 1], F32)
    nc.sync.dma_start(out=win, in_=ins["win"])
    nc.sync.dma_start(out=wa, in_=ins["wa"])
    nc.sync.dma_start(out=wb, in_=ins["wb"])
    nc.sync.dma_start(out=onesd, in_=ins["onesd"])
    nc.sync.dma_start(out=ident, in_=ins["ident"])
    nc.sync.dma_start(out=wout, in_=ins["wout"])
    nc.sync.dma_start(out=percol, in_=ins["percol"])
    nc.vector.memset(epsc, EPS)

    bcol = [percol[:, l: l + 1] for l in range(7)]           # stage 0..6 bias
    gcol = [percol[:, 7 + l: 8 + l] for l in range(NL)]
    tcol = [percol[:, 13 + l: 14 + l] for l in range(NL)]    # beta

    # ---- zero-padded x in DRAM ----
    zrow = singles.tile([1, 4], R)
    nc.vector.memset(zrow.bitcast(F32), 0.0)
    for r0 in range(4):
        for xp in (xpad_r, xpad_i):
            nc.sync.dma_start(out=xp[r0: r0 + 1, 0:3], in_=zrow[0:1, 0:3])
    nc.sync.dma_start(out=xpad_r[:, 3:], in_=xr)
    nc.sync.dma_start(out=xpad_i[:, 3:], in_=xi)

    # ---------------- main loops ----------------
    def load_feats(rowA, s):
        fts = []
        for g in range(NG):
            t0 = (s * SUP + 2 * g) * CH
            ft = ftp.tile([16, GW], R, tag="feats", name=f"ft_s{s}g{g}")
            for (pbase, xp) in ((0, xpad_r), (8, xpad_i)):
                src = bass.AP(
                    tensor=xp.tensor,
                    offset=rowA * (TPR + 3) + t0,
                    ap=[[2 * (TPR + 3), 2], [1, 4], [1, GW]])
                nc.sync.dma_start(out=ft[pbase: pbase + 8, :], in_=src)
            fts.append(ft)
        return fts

    def emit_dense(C, l, g):
        C["zps"][g] = zpp.tile([128, GW], F32, tag="zp",
                               name=f"zp_l{l}g{g}")
        zp = C["zps"][g]
        if l == 0:
            for h in range(2):
                nc.tensor.matmul(
                    out=zp[:, h * CH:(h + 1) * CH],
                    lhsT=win, rhs=C["feats"][g][:, h * CH:(h + 1) * CH],
                    start=True, stop=True)
            return
        inj = l in (2, 4, 6)
        for h in range(2):
            nc.tensor.matmul(
                out=zp[:, h * CH:(h + 1) * CH],
                lhsT=wb[:, (l - 1) * 128: l * 128],
                rhs=C["y_cur"][g][:, h * CH:(h + 1) * CH],
                start=True, stop=False)
        for h in range(2):
            nc.tensor.matmul(
                out=zp[:, h * CH:(h + 1) * CH],
                lhsT=wa[:, (l - 1) * 128: l * 128],
                rhs=C["r_cur"][g][:, h * CH:(h + 1) * CH],
                start=False, stop=not inj)
        if inj:                     # residual: TRUE-value add on PE
            for h in range(2):
                nc.tensor.matmul(
                    out=zp[:, h * CH:(h + 1) * CH],
                    lhsT=ident,
                    rhs=C["vb_src"][g][:, h * CH:(h + 1) * CH],
                    start=False, stop=True)

    def emit_vb(C, l, g):
        true_stage = l in TRUE_STAGES
        pool = vbsp if true_stage else vbtp
        vb = pool.tile([128, GW], BF, tag="vbs" if true_stage else "vbt",
                       name=f"vb_l{l}g{g}")
        nc.scalar.activation(out=vb, in_=C["zps"][g], func=AF.Identity,
                             bias=bcol[l], scale=1.0)
        if true_stage:
            C["vb_src"][g] = vb
        C["vbs"][g] = vb

    def emit_mid(C, l, g):
        """mu matmuls + centering sub (TRUE stages) and the square."""
        if l in TRUE_STAGES:
            mu0 = mup.tile([128, CH], F32, tag="mu")
            mu1 = mup.tile([128, CH], F32, tag="mu")
            nc.tensor.matmul(out=mu0, lhsT=onesd, rhs=C["vbs"][g][:, 0:CH],
                             start=True, stop=True)
            nc.tensor.matmul(out=mu1, lhsT=onesd, rhs=C["vbs"][g][:, CH:GW],
                             start=True, stop=True)
            v = vbtp.tile([128, GW], BF, tag="vbt", name=f"v_l{l}g{g}")
            nc.vector.tensor_sub(v[:, 0:CH], C["vbs"][g][:, 0:CH], mu0)
            nc.vector.tensor_sub(v[:, CH:GW], C["vbs"][g][:, CH:GW], mu1)
        else:
            v = C["vbs"][g]
        C["vs"][g] = v
        sq = sqp.tile([128, GW], BF, tag="sq", name=f"sq_l{l}g{g}")
        if (l + g) % 2 == 0:        # Pool is SBUF-only; give it half the sq
            nc.gpsimd.tensor_mul(sq, v, v)
        else:
            nc.vector.tensor_mul(sq, v, v)
        C["sqs"][g] = sq

    def emit_tail(C, l, g):
        va0 = vap.tile([128, CH], F32, tag="va")
        va1 = vap.tile([128, CH], F32, tag="va")
        nc.tensor.matmul(out=va0, lhsT=onesd, rhs=C["sqs"][g][:, 0:CH],
                         start=True, stop=True)
        nc.tensor.matmul(out=va1, lhsT=onesd, rhs=C["sqs"][g][:, CH:GW],
                         start=True, stop=True)
        rs = rsp.tile([128, GW], BF, tag="rs", name=f"rs_l{l}g{g}")
        nc.scalar.activation(out=rs[:, 0:CH], in_=va0,
                             func=AF.Abs_reciprocal_sqrt,
                             bias=epsc, scale=1.0)
        nc.scalar.activation(out=rs[:, CH:GW], in_=va1,
                             func=AF.Abs_reciprocal_sqrt,
                             bias=epsc, scale=1.0)
        xh = xhp.tile([128, GW], BF, tag="xh", name=f"xh_l{l}g{g}")
        nc.vector.tensor_mul(xh, C["vs"][g], rs)
        # y = gamma*xh + beta ; r = max(y, 0)
        yt = ytp.tile([128, GW], BF, tag="yt", name=f"y_l{l}g{g}")
        rt = ytp.tile([128, GW], BF, tag="rt", name=f"r_l{l}g{g}")
        nc.vector.tensor_scalar(out=yt, in0=xh,
                                scalar1=gcol[l], scalar2=tcol[l],
                                op0=ALU.mult, op1=ALU.add)
        nc.vector.tensor_scalar_max(rt, yt, 0.0)
        C["y_cur"][g], C["r_cur"][g] = yt, rt

    def emit_final(C, g):
        ot = otp.tile([4, GW], F32, tag="ot", name=f"ot_g{g}")
        for h in range(2):
            sl = slice(h * CH, (h + 1) * CH)
            opt_ = vap.tile([4, CH], F32, tag="va",
                            padded_shape=[128, CH],
                            name=f"op_g{g}h{h}")
            nc.tensor.matmul(out=opt_, lhsT=wout, rhs=C["vbs"][g][:, sl],
                             start=True, stop=True)
            nc.vector.tensor_copy(ot[:, sl], opt_)
        t0 = (C["s"] * SUP + 2 * g) * CH
        for (hp, row) in ((0, C["rowA"]), (1, C["rowA"] + 2)):
            dst = bass.AP(
                tensor=out.tensor,
                offset=row * TPR * 2 + t0 * 2,
                ap=[[1, 2], [2, GW]])
            nc.sync.dma_start(out=dst, in_=ot[2 * hp: 2 * hp + 2, :])

    flat = [(rp, s) for rp in range(2) for s in range(spr)]
    feats_next = load_feats(*flat[0])

    for idx, (rowA, s) in enumerate(flat):
        C = dict(rowA=rowA, s=s, feats=feats_next,
                 vb_src=[None] * NG, vbs=[None] * NG, vs=[None] * NG,
                 sqs=[None] * NG, y_cur=[None] * NG, r_cur=[None] * NG,
                 zps=[None] * NG)
        # 2-deep software pipeline: ACT sees vb(g), vb(g+1) before rs(g);
        # PE sees dense(g+1) before mu(g)/va(g-1).
        for l in range(NL + 1):         # stages 0..6
            if l == 4 and idx + 1 < len(flat):
                feats_next = load_feats(*flat[idx + 1])
            for g in range(NG):
                emit_dense(C, l, g)
                emit_vb(C, l, g)
                if l == NL:
                    emit_final(C, g)
                    continue
                if g >= 1:
                    emit_mid(C, l, g - 1)
                if g >= 2:
                    emit_tail(C, l, g - 2)
            if l < NL:
                emit_mid(C, l, NG - 1)
                emit_tail(C, l, NG - 2)
                emit_tail(C, l, NG - 1)
    ctx.close()


def _host_pack(inputs):
    """Build the shared (replicated) packed-weight arrays."""
    bf = mybir.dt.np(BF)
    w_in = np.asarray(inputs["w_in"], np.float32)        # [8, 64]
    dense_w = np.asarray(inputs["dense_w"], np.float32)  # [6, 64, 64]
    w_out = np.asarray(inputs["w_out"], np.float32)      # [64, 2]
    ln_gamma = np.asarray(inputs["ln_gamma"], np.float32)
    ln_beta = np.asarray(inputs["ln_beta"], np.float32)
    alpha = np.asarray(inputs["alpha"], np.float32)
    b_in = np.asarray(inputs["b_in"], np.float32)
    dense_b = np.asarray(inputs["dense_b"], np.float32)
    b_out = np.asarray(inputs["b_out"], np.float32)

    # stage l (1..6) uses dense_w[l-1]; stages 2,4,6 (weights 1,3,5) stay
    # TRUE (uncentered); stages 1,3,5 (weights 0,2,4) are centered.
    dw = dense_w.copy()
    db = dense_b.copy()
    for wi in (0, 2, 4):
        dw[wi] = dw[wi] - dw[wi].mean(axis=1, keepdims=True)
        db[wi] = db[wi] - db[wi].mean()

    win = np.zeros((16, 128), np.float32)   # feats: [A-re, B-re, A-im, B-im]
    win[0:4, 0:64] = w_in[0:4]
    win[4:8, 64:128] = w_in[0:4]
    win[8:12, 0:64] = w_in[4:8]
    win[12:16, 64:128] = w_in[4:8]

    wa = np.zeros((128, NL * 128), np.float32)
    wb = np.zeros((128, NL * 128), np.float32)
    for l in range(NL):
        Wl = dw[l]
        a = alpha[l][:, None]
        wa[0:64, l * 128: l * 128 + 64] = (1.0 - a) * Wl
        wa[64:128, l * 128 + 64: l * 128 + 128] = (1.0 - a) * Wl
        wb[0:64, l * 128: l * 128 + 64] = a * Wl
        wb[64:128, l * 128 + 64: l * 128 + 128] = a * Wl

    onesd = np.zeros((128, 128), np.float32)
    onesd[0:64, 0:64] = 1.0 / F
    onesd[64:128, 64:128] = 1.0 / F
    ident = np.eye(128, dtype=np.float32)

    wout = np.zeros((128, 4), np.float32)
    wout[0:64, 0:2] = w_out
    wout[64:128, 2:4] = w_out

    percol = np.zeros((128, 19), np.float32)
    percol[:, 0] = np.tile(b_in, 2)
    for l in range(NL):
        percol[:, 1 + l] = np.tile(db[l], 2)
        percol[:, 7 + l] = np.tile(ln_gamma[l], 2)
        percol[:, 13 + l] = np.tile(ln_beta[l], 2)

    return dict(win=win, wa=wa.astype(bf), wb=wb.astype(bf),
                onesd=onesd.astype(bf), ident=ident.astype(bf),
                wout=wout.astype(bf), percol=percol), b_out


def build_program(tokens_per_row):
    nc = bacc.Bacc("TRN2")
    ins = {}
    shapes = dict(win=((16, 128), R), wa=((128, NL * 128), BF),
                  wb=((128, NL * 128), BF), onesd=((128, 128), BF),
                  ident=((128, 128), BF), wout=((128, 4), BF),
                  percol=((128, 19), F32))
    for name, (shp, dt) in shapes.items():
        ins[name] = nc.dram_tensor(name, list(shp), dt, kind="ExternalInput").ap()
    ins["xr"] = nc.dram_tensor("xr", [4, tokens_per_row], R,
                               kind="ExternalInput").ap()
    ins["xi"] = nc.dram_tensor("xi", [4, tokens_per_row], R,
                               kind="ExternalInput").ap()
    outs = {"out": nc.dram_tensor("out", [4, tokens_per_row, 2], F32,
                                  kind="ExternalOutput").ap()}
    with tile.TileContext(nc) as tc:
        build_kernel(tc, outs, ins, tokens_per_row)
    nc.compile()
    return nc


def _run(inputs, trace=False):
    from concourse.bass_utils import run_bass_kernel_spmd

    x_real = np.asarray(inputs["x_real"], np.float32)
    x_imag = np.asarray(inputs["x_imag"], np.float32)
    B, N = x_real.shape
    n_cores = 8
    rows_per_core = B // n_cores

    shared, b_out_eff = _host_pack(inputs)
    nc = build_program(N)

    in_maps = []
    for c in range(n_cores):
        m = dict(shared)
        m["xr"] = np.ascontiguousarray(x_real[c * rows_per_core:(c + 1) * rows_per_core])
        m["xi"] = np.ascontiguousarray(x_imag[c * rows_per_core:(c + 1) * rows_per_core])
        in_maps.append(m)

    res = run_bass_kernel_spmd(nc, in_maps, core_ids=list(range(n_cores)),
                               trace=trace)
    outs_np = [r["out"] for r in res.results]
    full = np.concatenate(outs_np, axis=0)          # [B, N, 2]
    re = full[..., 0] + b_out_eff[0] + x_real
    im = full[..., 1] + b_out_eff[1] + x_imag
    return (re + 1j * im).astype(np.complex64), res


def kernel(**inputs):
    return _run(inputs, trace=False)[0]
